# revision 3
# baseline (speedup 1.0000x reference)
"""TV-Chambolle denoise (weight=0.1, eps=2e-4, n_iter_max=200) on 8 Trainium2
NeuronCores via Bass/Tile.

Sharding: embarrassingly parallel over channels — core c solves channel c%3
(cores 3-7 run duplicates; host reads cores 0-2).

Layout per channel: 512x512 fp32 image in "strip" layout [128, 4*512]:
partition p holds rows 4p..4p+3 contiguously (C-order reshape(128, 2048)).
H-direction stencil shifts are free-dim offsets for 3/4 of rows; the 127
strip-boundary rows use SBUF->SBUF DMA halo copies with partition remap.

Early stopping: the reference freezes its state once |E_prev-E| < eps*E_init.
On device this is done with per-partition scalar tau_eff = tau*(1-done) where
done incorporates the CURRENT iteration's convergence flag: p then freezes at
the conv iteration i*, so t = img + div(p_{i*}) equals the reference's output
with no plane-level selects. The kernel runs K=25 iterations per launch and
outputs (t, p0, p1, scalars); the host relaunches (up to 200 total iterations)
only if some channel has not converged. The reference input converges at
iteration 21, so one launch suffices.
"""
import sys
if '/opt/trn_rl_repo' not in sys.path:
    sys.path.insert(0, '/opt/trn_rl_repo')

import numpy as np

F32_EPS = 2e-4
WEIGHT = 0.1
TAU = 0.25
P, J, W = 128, 4, 512
FREE = J * W
K_CHUNK = 25
N_ITER_MAX = 200
N_CORES = 8

_NC = None
LAST_RESULTS = []


def _build():
    import concourse.bacc as bacc
    import concourse.tile as tile
    import concourse.mybir as mybir
    from concourse import bass_isa
    from contextlib import ExitStack

    F32 = mybir.dt.float32
    ALU = mybir.AluOpType
    ACTF = mybir.ActivationFunctionType
    K = K_CHUNK

    nc = bacc.Bacc('TRN2', target_bir_lowering=False, debug=False)

    img_d = nc.declare_dram_parameter("img", [P, FREE], F32, isOutput=False)
    p0_d = nc.declare_dram_parameter("p0_in", [P, FREE], F32, isOutput=False)
    p1_d = nc.declare_dram_parameter("p1_in", [P, FREE], F32, isOutput=False)
    scal_d = nc.declare_dram_parameter("scal_in", [P, 4], F32, isOutput=False)
    out_d = nc.declare_dram_parameter("out_t", [P, FREE], F32, isOutput=True)
    p0o_d = nc.declare_dram_parameter("p0_out", [P, FREE], F32, isOutput=True)
    p1o_d = nc.declare_dram_parameter("p1_out", [P, FREE], F32, isOutput=True)
    scalo_d = nc.declare_dram_parameter("scal_out", [P, 4], F32, isOutput=True)

    with tile.TileContext(nc) as tc, ExitStack() as ctx:
        pool = ctx.enter_context(tc.tile_pool(name="st", bufs=1))

        def T(name, shape=(P, FREE)):
            return pool.tile(list(shape), F32, name=name, tag=name)

        img = T("img_t"); p0 = T("p0"); p1 = T("p1")
        dneg = T("dneg"); Bp = T("Bp"); t = T("t")
        g0 = T("g0"); g1 = T("g1")
        sq0 = T("sq0"); n2 = T("n2")
        denom = T("den"); r = T("r"); rs = T("rs")
        u0 = T("u0"); u1 = T("u1")
        scr = T("scr")
        halo_p = T("halo_p", (P, W)); halo_t = T("halo_t", (P, W))
        scal = T("scal", (P, 4))
        Ed = T("Ed", (P, 1)); En = T("En", (P, 1)); c_ = T("c", (P, 1))
        Es = T("Es", (P, 1)); dE = T("dE", (P, 1)); th = T("th", (P, 1))
        conv = T("conv", (P, 1)); nfirst = T("nf", (P, 1))
        notdone = T("nd", (P, 1)); s_u = T("s_u", (P, 1)); s_ow = T("s_ow", (P, 1))
        tmp1 = T("tmp1", (P, 1)); tmp2 = T("tmp2", (P, 1))

        E_prev = scal[:, 0:1]; E_init = scal[:, 1:2]
        done = scal[:, 2:3]; first = scal[:, 3:4]

        nc.sync.dma_start(img[:], img_d.ap())
        nc.sync.dma_start(p0[:], p0_d.ap())
        nc.sync.dma_start(p1[:], p1_d.ap())
        nc.sync.dma_start(scal[:], scal_d.ap())

        nc.vector.memset(halo_p[:], 0.0)
        nc.vector.memset(halo_t[:], 0.0)
        nc.vector.memset(g0[:], 0.0)
        nc.vector.memset(g1[:], 0.0)
        nc.vector.tensor_scalar(nfirst[:], first[:], -1.0, 1.0, ALU.mult, ALU.add)
        nc.sync.dma_start(halo_p[1:128, :], p0[0:127, 3 * W:4 * W])

        def v3(ap):
            return ap.rearrange("p (j w) -> p j w", w=W)

        for j in range(K):
            # B' = p1 - shiftW(p1)  (GPSIMD, off the DVE critical path)
            Bp3 = v3(Bp[:]); p13 = v3(p1[:])
            nc.gpsimd.tensor_copy(Bp3[:, :, 0:1], p13[:, :, 0:1])
            nc.gpsimd.tensor_tensor(Bp3[:, :, 1:W], p13[:, :, 1:W], p13[:, :, 0:W - 1], ALU.subtract)

            # A = p0 - shiftH(p0) into dneg (DVE); then dneg = A + B'
            nc.vector.tensor_copy(dneg[:], p0[:])
            d3 = v3(dneg[:]); p03 = v3(p0[:])
            nc.vector.tensor_tensor(d3[:, 1:4, :], d3[:, 1:4, :], p03[:, 0:3, :], ALU.subtract)
            nc.vector.tensor_tensor(d3[:, 0, :], d3[:, 0, :], halo_p[:, :], ALU.subtract)
            nc.vector.tensor_add(dneg[:], dneg[:], Bp[:])

            # t = img - dneg  (dneg == -div(p))
            nc.vector.tensor_sub(t[:], img[:], dneg[:])
            nc.sync.dma_start(halo_t[0:127, :], t[1:128, 0:W])

            # Ed = sum(dneg^2) per partition
            nc.scalar.activation(scr[:], dneg[:], ACTF.Square, accum_out=Ed[:])

            # gradients
            t3 = v3(t[:]); g03 = v3(g0[:]); g13 = v3(g1[:])
            nc.vector.tensor_tensor(g03[:, 0:3, :], t3[:, 1:4, :], t3[:, 0:3, :], ALU.subtract)
            nc.vector.tensor_tensor(g03[0:127, 3, :], halo_t[0:127, :], t3[0:127, 3, :], ALU.subtract)
            nc.gpsimd.tensor_tensor(g13[:, :, 0:W - 1], t3[:, :, 1:W], t3[:, :, 0:W - 1], ALU.subtract)

            # n2 = g0^2 + g1^2 ; norm = sqrt(n2) with En = sum(norm)
            nc.scalar.activation(sq0[:], g0[:], ACTF.Square)
            nc.scalar.activation(n2[:], g1[:], ACTF.Square)
            nc.gpsimd.tensor_add(n2[:], n2[:], sq0[:])
            nc.scalar.activation(n2[:], n2[:], ACTF.Sqrt, accum_out=En[:])
            norm = n2

            # denom = 1 + (tau/w)*norm with CONSTANT scale (no E dependency) so
            # the reciprocal overlaps the convergence-scalar chain; the freeze is
            # applied to r instead: r_eff = r*notdone + done (exactly 1.0 when done).
            nc.scalar.activation(denom[:], norm[:], ACTF.Identity, bias=1.0,
                                 scale=float(TAU / WEIGHT))
            nc.vector.reciprocal_approx_accurate(r[:], denom[:], rs[:])

            # E chain; E kept raw (x size) — the convergence test is scale-invariant
            nc.vector.scalar_tensor_tensor(c_[:], En[:], WEIGHT, Ed[:], ALU.mult, ALU.add)
            nc.gpsimd.partition_all_reduce(Es[:], c_[:], 128, bass_isa.ReduceOp.add)
            if j == 0:
                nc.vector.tensor_mul(tmp1[:], Es[:], first[:])
                nc.vector.tensor_mul(tmp2[:], E_init, nfirst[:])
                nc.vector.tensor_add(E_init, tmp1[:], tmp2[:])
            nc.vector.tensor_sub(dE[:], E_prev, Es[:])
            nc.scalar.activation(dE[:], dE[:], ACTF.Abs)
            nc.vector.tensor_scalar(th[:], E_init, float(F32_EPS), None, ALU.mult)
            nc.vector.tensor_tensor(conv[:], dE[:], th[:], ALU.is_lt)
            nc.vector.tensor_tensor(done, done, conv[:], ALU.max)
            nc.vector.tensor_copy(E_prev, Es[:])
            nc.vector.tensor_scalar(notdone[:], done, -1.0, 1.0, ALU.mult, ALU.add)
            nc.vector.tensor_scalar(s_u[:], notdone[:], float(-TAU), None, ALU.mult)

            # r_eff = r*notdone + done (2x-mode tensor_scalar)
            nc.vector.tensor_scalar(r[:], r[:], notdone[:], done, ALU.mult, ALU.add)

            # p update: p = (p - tau_eff*g) * r_eff; p1 first so the next
            # iteration's GPSIMD W-shift (which needs only p1) starts early.
            nc.vector.scalar_tensor_tensor(u1[:], g1[:], s_u[:], p1[:], ALU.mult, ALU.add)
            nc.vector.tensor_mul(p1[:], u1[:], r[:])
            nc.vector.scalar_tensor_tensor(u0[:], g0[:], s_u[:], p0[:], ALU.mult, ALU.add)
            nc.vector.tensor_mul(p0[:], u0[:], r[:])

            if j + 1 < K:
                nc.sync.dma_start(halo_p[1:128, :], p0[0:127, 3 * W:4 * W])

        nc.sync.dma_start(out_d.ap(), t[:])
        nc.sync.dma_start(p0o_d.ap(), p0[:])
        nc.sync.dma_start(p1o_d.ap(), p1[:])
        nc.sync.dma_start(scalo_d.ap(), scal[:])

    nc.compile()
    return nc


def _get_nc():
    global _NC
    if _NC is None:
        _NC = _build()
    return _NC


def kernel(img: np.ndarray) -> np.ndarray:
    from concourse.bass_utils import run_bass_kernel_spmd

    assert img.shape == (3, 512, 512) and img.dtype == np.float32
    nc = _get_nc()
    del LAST_RESULTS[:]

    core_ids = list(range(N_CORES))
    p0s = [np.zeros((P, FREE), np.float32) for _ in core_ids]
    p1s = [np.zeros((P, FREE), np.float32) for _ in core_ids]
    scals = []
    for c in core_ids:
        s = np.zeros((P, 4), np.float32)
        s[:, 3] = 1.0  # first chunk
        scals.append(s)
    imgs = [np.ascontiguousarray(img[c % 3].reshape(P, FREE)) for c in core_ids]

    iters = 0
    outs = None
    while iters < N_ITER_MAX:
        in_maps = [
            {"img": imgs[c], "p0_in": p0s[c], "p1_in": p1s[c], "scal_in": scals[c]}
            for c in core_ids
        ]
        res = run_bass_kernel_spmd(nc, in_maps, core_ids)
        LAST_RESULTS.append(res)
        iters += K_CHUNK
        outs = res.results
        if all(outs[c]["scal_out"][0, 2] > 0.5 for c in range(3)):
            break
        for c in core_ids:
            p0s[c] = outs[c]["p0_out"]
            p1s[c] = outs[c]["p1_out"]
            s = outs[c]["scal_out"].copy()
            s[:, 3] = 0.0  # no longer the first chunk
            scals[c] = s

    result = np.empty((3, 512, 512), np.float32)
    for c in range(3):
        result[c] = outs[c]["out_t"].reshape(512, 512)
    return result
